# revision 22
# baseline (speedup 1.0000x reference)
"""AttentionBlock kernel for 8 Trainium2 NeuronCores.

Sharding: one (batch, head) pair per core (B=2 x H=4 = 8 cores).
Each core computes, for its (b, h):
    qT/kT = (w_q/k^T @ x_b) + bias        [64, S]   (S padded to 2816)
    v     = x_b^T @ w_v                   [S, 64]  (+ ones column -> [S, 65])
    S^T[j, i] = sum_d k[j,d] q[i,d]                (22 j-tiles of 128)
    E = exp(S^T * 0.125 - 3)                       (ScalarE, from PSUM)
    resT[d, i] = sum_j v_aug[j, d] E[j, i]         (PSUM accumulation, 65 rows;
                                                    row 64 = softmax denominator l)
    outT[c, i] = sum_d w_out[d, c] resT[d, i]      [256, S] (unnormalized)
Host: out_b = sum_h (outT / l + (b_v @ w_out_h)[:, None]) + b_out[:, None] + x_b.

The -3 bias and missing max-subtraction cancel in the normalization; score
scale is 1/sqrt(64) = 0.125.

Matmul operands use float32r (single-pass PE fp32, ~TF32 precision, ~3x the
throughput of 2-pass fp32); PSUM accumulation stays fp32.
"""

import numpy as np

C = 256
S = 2744
SP = 2816  # 22 * 128
H = 4
DK = 64
NT = 22  # j tiles of 128
SVALID_LAST = S - 21 * 128  # 56 valid rows in last j-tile

# i blocks (query positions): only valid range [0, 2744)
IBLOCKS = [(0, 512), (512, 512), (1024, 512), (1536, 512), (2048, 512), (2560, 184)]
# s blocks for the qk projection: full padded range [0, 2816)
SBLOCKS = [(0, 512), (512, 512), (1024, 512), (1536, 512), (2048, 512), (2560, 256)]

_NC = None
PACK_SCORES = True


def _build():
    from contextlib import ExitStack

    import concourse.bacc as bacc
    import concourse.tile as tile
    from concourse import mybir

    f32 = mybir.dt.float32
    fr = mybir.dt.float32r
    f16 = mybir.dt.float16
    Exp = mybir.ActivationFunctionType.Exp

    nc = bacc.Bacc("TRN2", target_bir_lowering=False)

    xT = nc.dram_tensor("xT", [C, S], f16, kind="ExternalInput")
    wq = nc.dram_tensor("wq", [C, DK], f16, kind="ExternalInput")
    wk = nc.dram_tensor("wk", [C, DK], f16, kind="ExternalInput")
    wv = nc.dram_tensor("wv", [C, DK], f16, kind="ExternalInput")
    bq = nc.dram_tensor("bq", [128, 1], f32, kind="ExternalInput")
    bk = nc.dram_tensor("bk", [128, 1], f32, kind="ExternalInput")
    wo = nc.dram_tensor("wo", [DK, C], f32, kind="ExternalInput")

    out = nc.dram_tensor("out", [C, S], f32, kind="ExternalOutput")
    lsum = nc.dram_tensor("lsum", [1, S], f32, kind="ExternalOutput")

    with tile.TileContext(nc) as tc, ExitStack() as ctx:
        consts = ctx.enter_context(tc.tile_pool(name="consts", bufs=1))
        big = ctx.enter_context(tc.tile_pool(name="big", bufs=1))
        expp = ctx.enter_context(tc.tile_pool(name="expp", bufs=3))
        resp = ctx.enter_context(tc.tile_pool(name="resp", bufs=2))
        outp = ctx.enter_context(tc.tile_pool(name="outp", bufs=2))
        scp = ctx.enter_context(tc.tile_pool(name="scp", bufs=2, space="PSUM"))
        psp = ctx.enter_context(tc.tile_pool(name="psp", bufs=4, space="PSUM"))

        # ---- weights / constants in SBUF (fp16 direct) ----
        w_sb = consts.tile([128, 2, 3 * DK], f16)
        for idx, w_dram in enumerate((wq, wk, wv)):
            nc.gpsimd.dma_start(
                out=w_sb[:, :, idx * DK : (idx + 1) * DK],
                in_=w_dram.rearrange("(c p) d -> p c d", p=128),
            )

        def wslice(idx, cc):
            return w_sb[:, cc, idx * DK : (idx + 1) * DK]

        wo_stage = consts.tile([DK, C], f32)
        nc.gpsimd.dma_start(out=wo_stage, in_=wo[:, :])
        wo_sb = consts.tile([DK, C], fr)
        nc.vector.tensor_copy(wo_sb, wo_stage)

        bq_sb = consts.tile([128, 1], f32)
        nc.gpsimd.dma_start(out=bq_sb, in_=bq[:, :])
        bk_sb = consts.tile([128, 1], f32)
        nc.gpsimd.dma_start(out=bk_sb, in_=bk[:, :])
        ebias_sb = consts.tile([128, 1], f32)
        nc.vector.memset(ebias_sb, -3.0)

        # ---- x in SBUF (fp16 direct) ----
        x_sb = big.tile([128, 2, SP], f16)
        nc.vector.memset(x_sb[:, :, S:SP], 0.0)
        for off, w in SBLOCKS:
            for cc in range(2):
                wv_ = min(w, S - off) if off < S else 0
                if wv_ > 0:
                    eng = nc.sync if cc == 0 else nc.gpsimd
                    eng.dma_start(
                        out=x_sb[:, cc, off : off + wv_],
                        in_=xT[cc * 128 : (cc + 1) * 128, off : off + wv_],
                    )

        # ---- q/k/v projections, emitted as chunks interleaved into i-block 0 ----
        qT_sb = big.tile([128, SP], f16)
        kT_sb = big.tile([128, SP], f16)
        v_sb = big.tile([128, NT, DK + 1], f16)
        nc.vector.memset(v_sb[:, : NT - 1, DK : DK + 1], 1.0)
        nc.vector.memset(v_sb[:, NT - 1, DK : DK + 1], 0.0)
        nc.vector.memset(v_sb[:SVALID_LAST, NT - 1, DK : DK + 1], 1.0)

        def qk_chunk(sb):
            off, w = SBLOCKS[sb]
            ps = psp.tile([128, 512], f32, tag="ps", name="psqk")
            for cc in range(2):
                nc.tensor.matmul(
                    ps[:, :w],
                    lhsT=w_sb[:, cc, : 2 * DK],
                    rhs=x_sb[:, cc, off : off + w],
                    start=(cc == 0),
                    stop=(cc == 1),
                )
            nc.vector.tensor_scalar_add(
                qT_sb[:DK, off : off + w], ps[:DK, :w], bq_sb[:DK]
            )
            nc.vector.tensor_scalar_add(
                kT_sb[DK:, off : off + w], ps[DK:, :w], bk_sb[DK:]
            )
            nc.gpsimd.dma_start(
                out=qT_sb[DK:, off : off + w], in_=qT_sb[:DK, off : off + w]
            )
            nc.gpsimd.dma_start(
                out=kT_sb[:DK, off : off + w], in_=kT_sb[DK:, off : off + w]
            )

        def v_chunk(p):
            for t in (2 * p, 2 * p + 1):
                psv = psp.tile([128, DK], f32, tag="ps", name="psv")
                for cc in range(2):
                    nc.tensor.matmul(
                        psv,
                        lhsT=x_sb[:, cc, t * 128 : (t + 1) * 128],
                        rhs=wslice(2, cc),
                        start=(cc == 0),
                        stop=(cc == 1),
                    )
                nc.vector.tensor_copy(v_sb[:, t, :DK], psv)

        for sb in range(6):
            qk_chunk(sb)
        chunk_map = {
            g: [
                (lambda p=2 * g: v_chunk(p)),
                *([(lambda p=2 * g + 1: v_chunk(p))] if 2 * g + 1 < NT // 2 else []),
            ]
            for g in range(6)
        }

        # ---- main attention loop ----
        NG = NT // 2  # groups of 2 j-tiles per exp op

        def emit_pv(pv, pex, pg, iw):
            for u in range(2):
                t = 2 * pg + u
                nc.tensor.matmul(
                    pv[:, :iw],
                    lhsT=v_sb[:, t, :],
                    rhs=pex[:, u * 512 : u * 512 + iw],
                    start=(t == 0),
                    stop=(t == NT - 1),
                )

        pending_tail = None
        for ioff, iw in IBLOCKS:
            pv = psp.tile([DK + 1, 512], f32, tag="ps", name="pv")
            pending_pv = []
            for g in range(NG):
                if ioff == 0:
                    for ck in chunk_map.get(g, ()):
                        ck()
                sc = scp.tile([128, 1024], f32, tag="sc", name="sc")
                for u in range(2):
                    t = 2 * g + u
                    lo, hi = (u * DK, (u + 1) * DK) if PACK_SCORES else (0, DK)
                    nc.tensor.matmul(
                        sc[:, u * 512 : u * 512 + iw],
                        lhsT=kT_sb[lo:hi, t * 128 : (t + 1) * 128],
                        rhs=qT_sb[lo:hi, ioff : ioff + iw],
                        start=True,
                        stop=True,
                        tile_position=(lo, 0),
                    )
                ex = expp.tile([128, 1024], f16, tag="ex", name="ex")
                sc3 = sc.rearrange("p (b w) -> p b w", b=2)[:, :, :iw]
                ex3 = ex.rearrange("p (b w) -> p b w", b=2)[:, :, :iw]
                nc.scalar.activation(
                    out=ex3,
                    in_=sc3,
                    func=Exp,
                    bias=ebias_sb,
                    scale=0.125,
                )
                if g == 1 and pending_tail is not None:
                    pending_tail()
                    pending_tail = None
                pending_pv.append((ex, g))
                if len(pending_pv) > 1:
                    pex, pg = pending_pv.pop(0)
                    emit_pv(pv, pex, pg, iw)
            for pex, pg in pending_pv:
                emit_pv(pv, pex, pg, iw)
            res_sb = resp.tile([DK + 1, 512], fr, tag="res", name="res_sb")
            nc.vector.tensor_copy(res_sb[:, :iw], pv[:, :iw])
            nc.gpsimd.dma_start(
                out=lsum[0:1, ioff : ioff + iw],
                in_=res_sb[DK : DK + 1, :iw].bitcast(f32),
            )

            def tail(ioff=ioff, iw=iw, res_sb=res_sb):
                for cc in range(2):
                    po = psp.tile([128, 512], f32, tag="ps", name="po")
                    nc.tensor.matmul(
                        po[:, :iw],
                        lhsT=wo_sb[:, cc * 128 : (cc + 1) * 128],
                        rhs=res_sb[:DK, :iw],
                        start=True,
                        stop=True,
                    )
                    ob = outp.tile([128, 512], f32, tag="ob", name="ob")
                    nc.vector.tensor_copy(ob[:, :iw], po[:, :iw])
                    nc.sync.dma_start(
                        out=out[cc * 128 : (cc + 1) * 128, ioff : ioff + iw],
                        in_=ob[:, :iw],
                    )

            pending_tail = tail
        pending_tail()

    nc.compile()
    return nc


def _get_nc():
    global _NC
    if _NC is None:
        _NC = _build()
    return _NC



def _make_in_maps(inputs):
    x = np.asarray(inputs["x"], dtype=np.float32)
    w_proj = np.asarray(inputs["w_proj"], dtype=np.float32)
    b_proj = np.asarray(inputs["b_proj"], dtype=np.float32)
    w_out = np.asarray(inputs["w_out"], dtype=np.float32)
    in_maps = []
    for core in range(8):
        b, h = divmod(core, H)
        base = h * 3 * DK
        in_maps.append(
            {
                "xT": np.ascontiguousarray(x[b].reshape(C, S).astype(np.float16)),
                "wq": np.ascontiguousarray(
                    w_proj[:, base : base + DK].astype(np.float16)
                ),
                "wk": np.ascontiguousarray(
                    w_proj[:, base + DK : base + 2 * DK].astype(np.float16)
                ),
                "wv": np.ascontiguousarray(
                    w_proj[:, base + 2 * DK : base + 3 * DK].astype(np.float16)
                ),
                "bq": np.ascontiguousarray(
                    np.tile(b_proj[base : base + DK], 2).reshape(128, 1)
                ),
                "bk": np.ascontiguousarray(
                    np.tile(b_proj[base + DK : base + 2 * DK], 2).reshape(128, 1)
                ),
                "wo": np.ascontiguousarray(w_out[h * DK : (h + 1) * DK, :]),
            }
        )
    return in_maps


def kernel(x, w_proj, b_proj, w_out, b_out):
    from concourse.bass_utils import run_bass_kernel_spmd

    x = np.asarray(x, dtype=np.float32)
    w_proj = np.asarray(w_proj, dtype=np.float32)
    b_proj = np.asarray(b_proj, dtype=np.float32)
    w_out = np.asarray(w_out, dtype=np.float32)
    b_out = np.asarray(b_out, dtype=np.float32)

    B = x.shape[0]
    nc = _get_nc()

    in_maps = _make_in_maps(
        {"x": x, "w_proj": w_proj, "b_proj": b_proj, "w_out": w_out, "b_out": b_out}
    )
    res = run_bass_kernel_spmd(nc, in_maps, list(range(8)))

    outs = np.zeros((B, C, S), dtype=np.float32)
    for b in range(B):
        acc = x[b].reshape(C, S).astype(np.float32) + b_out[:, None]
        for h in range(H):
            core = b * H + h
            dev_o = res.results[core]["out"]  # [C, S] unnormalized
            l = res.results[core]["lsum"]  # [1, S]
            bv = b_proj[h * 3 * DK + 2 * DK : h * 3 * DK + 3 * DK]
            corr = bv @ w_out[h * DK : (h + 1) * DK, :]  # [C]
            acc = acc + dev_o / l + corr[:, None]
        outs[b] = acc
    return outs.reshape(B, C, 14, 14, 14)


# revision 23
# speedup vs baseline: 1.0355x; 1.0355x over previous
"""AttentionBlock kernel for 8 Trainium2 NeuronCores.

Sharding: one (batch, head) pair per core (B=2 x H=4 = 8 cores).
Each core computes, for its (b, h):
    qT/kT = (w_q/k^T @ x_b) + bias        [64, S]   (S padded to 2816)
    v     = x_b^T @ w_v                   [S, 64]  (+ ones column -> [S, 65])
    S^T[j, i] = sum_d k[j,d] q[i,d]                (22 j-tiles of 128)
    E = exp(S^T * 0.125 - 3)                       (ScalarE, from PSUM)
    resT[d, i] = sum_j v_aug[j, d] E[j, i]         (PSUM accumulation, 65 rows;
                                                    row 64 = softmax denominator l)
    outT[c, i] = sum_d w_out[d, c] resT[d, i]      [256, S] (unnormalized)
Host: out_b = sum_h (outT / l + (b_v @ w_out_h)[:, None]) + b_out[:, None] + x_b.

The -3 bias and missing max-subtraction cancel in the normalization; score
scale is 1/sqrt(64) = 0.125.

Matmul operands use float32r (single-pass PE fp32, ~TF32 precision, ~3x the
throughput of 2-pass fp32); PSUM accumulation stays fp32.
"""

import numpy as np

C = 256
S = 2744
SP = 2816  # 22 * 128
H = 4
DK = 64
NT = 22  # j tiles of 128
SVALID_LAST = S - 21 * 128  # 56 valid rows in last j-tile

# i blocks (query positions): only valid range [0, 2744)
IBLOCKS = [(0, 512), (512, 512), (1024, 512), (1536, 512), (2048, 512), (2560, 184)]
# s blocks for the qk projection: full padded range [0, 2816)
SBLOCKS = [(0, 512), (512, 512), (1024, 512), (1536, 512), (2048, 512), (2560, 256)]

_NC = None
PACK_SCORES = True


def _build():
    from contextlib import ExitStack

    import concourse.bacc as bacc
    import concourse.tile as tile
    from concourse import mybir

    f32 = mybir.dt.float32
    fr = mybir.dt.float32r
    f16 = mybir.dt.float16
    Exp = mybir.ActivationFunctionType.Exp

    nc = bacc.Bacc("TRN2", target_bir_lowering=False)

    xT = nc.dram_tensor("xT", [C, S], f16, kind="ExternalInput")
    wq = nc.dram_tensor("wq", [C, DK], f16, kind="ExternalInput")
    wk = nc.dram_tensor("wk", [C, DK], f16, kind="ExternalInput")
    wv = nc.dram_tensor("wv", [C, DK], f16, kind="ExternalInput")
    bq = nc.dram_tensor("bq", [128, 1], f32, kind="ExternalInput")
    bk = nc.dram_tensor("bk", [128, 1], f32, kind="ExternalInput")
    wo = nc.dram_tensor("wo", [DK, C], f32, kind="ExternalInput")

    out = nc.dram_tensor("out", [C, S], f32, kind="ExternalOutput")
    lsum = nc.dram_tensor("lsum", [1, S], f32, kind="ExternalOutput")

    with tile.TileContext(nc) as tc, ExitStack() as ctx:
        consts = ctx.enter_context(tc.tile_pool(name="consts", bufs=1))
        big = ctx.enter_context(tc.tile_pool(name="big", bufs=1))
        expp = ctx.enter_context(tc.tile_pool(name="expp", bufs=3))
        resp = ctx.enter_context(tc.tile_pool(name="resp", bufs=2))
        outp = ctx.enter_context(tc.tile_pool(name="outp", bufs=2))
        scp = ctx.enter_context(tc.tile_pool(name="scp", bufs=2, space="PSUM"))
        psp = ctx.enter_context(tc.tile_pool(name="psp", bufs=4, space="PSUM"))

        # ---- weights / constants in SBUF (fp16 direct) ----
        w_sb = consts.tile([128, 2, 3 * DK], f16)
        for idx, w_dram in enumerate((wq, wk, wv)):
            nc.gpsimd.dma_start(
                out=w_sb[:, :, idx * DK : (idx + 1) * DK],
                in_=w_dram.rearrange("(c p) d -> p c d", p=128),
            )

        def wslice(idx, cc):
            return w_sb[:, cc, idx * DK : (idx + 1) * DK]

        wo_stage = consts.tile([DK, C], f32)
        nc.gpsimd.dma_start(out=wo_stage, in_=wo[:, :])
        wo_sb = consts.tile([DK, C], fr)
        nc.vector.tensor_copy(wo_sb, wo_stage)

        bq_sb = consts.tile([128, 1], f32)
        nc.gpsimd.dma_start(out=bq_sb, in_=bq[:, :])
        bk_sb = consts.tile([128, 1], f32)
        nc.gpsimd.dma_start(out=bk_sb, in_=bk[:, :])
        ebias_sb = consts.tile([128, 1], f32)
        nc.vector.memset(ebias_sb, -3.0)

        # ---- x in SBUF (fp16 direct) ----
        x_sb = big.tile([128, 2, SP], f16)
        nc.vector.memset(x_sb[:, :, S:SP], 0.0)
        for off, w in SBLOCKS:
            for cc in range(2):
                wv_ = min(w, S - off) if off < S else 0
                if wv_ > 0:
                    eng = nc.sync if cc == 0 else nc.gpsimd
                    eng.dma_start(
                        out=x_sb[:, cc, off : off + wv_],
                        in_=xT[cc * 128 : (cc + 1) * 128, off : off + wv_],
                    )

        # ---- q/k/v projections, emitted as chunks interleaved into i-block 0 ----
        qT_sb = big.tile([128, SP], f16)
        kT_sb = big.tile([128, SP], f16)
        v_sb = big.tile([128, NT, DK + 1], f16)
        nc.vector.memset(v_sb[:, : NT - 1, DK : DK + 1], 1.0)
        nc.vector.memset(v_sb[:, NT - 1, DK : DK + 1], 0.0)
        nc.vector.memset(v_sb[:SVALID_LAST, NT - 1, DK : DK + 1], 1.0)

        def qk_chunk(sb):
            off, w = SBLOCKS[sb]
            for widx, dst, bias in ((0, qT_sb, bq_sb), (1, kT_sb, bk_sb)):
                ps = psp.tile([DK, 512], f32, tag="ps", name="psqk")
                for cc in range(2):
                    nc.tensor.matmul(
                        ps[:, :w],
                        lhsT=wslice(widx, cc),
                        rhs=x_sb[:, cc, off : off + w],
                        start=(cc == 0),
                        stop=(cc == 1),
                    )
                nc.vector.tensor_scalar_add(
                    dst[:DK, off : off + w], ps[:, :w], bias[:DK]
                )
                nc.gpsimd.dma_start(
                    out=dst[DK:, off : off + w], in_=dst[:DK, off : off + w]
                )

        def v_chunk(p):
            for t in (2 * p, 2 * p + 1):
                psv = psp.tile([128, DK], f32, tag="ps", name="psv")
                for cc in range(2):
                    nc.tensor.matmul(
                        psv,
                        lhsT=x_sb[:, cc, t * 128 : (t + 1) * 128],
                        rhs=wslice(2, cc),
                        start=(cc == 0),
                        stop=(cc == 1),
                    )
                nc.vector.tensor_copy(v_sb[:, t, :DK], psv)

        for sb in range(6):
            qk_chunk(sb)
        chunk_map = {
            g: [
                (lambda p=2 * g: v_chunk(p)),
                *([(lambda p=2 * g + 1: v_chunk(p))] if 2 * g + 1 < NT // 2 else []),
            ]
            for g in range(6)
        }

        # ---- main attention loop ----
        NG = NT // 2  # groups of 2 j-tiles per exp op

        def emit_pv(pv, pex, pg, iw):
            for u in range(2):
                t = 2 * pg + u
                nc.tensor.matmul(
                    pv[:, :iw],
                    lhsT=v_sb[:, t, :],
                    rhs=pex[:, u * 512 : u * 512 + iw],
                    start=(t == 0),
                    stop=(t == NT - 1),
                )

        pending_tail = None
        for ioff, iw in IBLOCKS:
            pv = psp.tile([DK + 1, 512], f32, tag="ps", name="pv")
            pending_pv = []
            for g in range(NG):
                if ioff == 0:
                    for ck in chunk_map.get(g, ()):
                        ck()
                sc = scp.tile([128, 1024], f32, tag="sc", name="sc")
                for u in range(2):
                    t = 2 * g + u
                    lo, hi = (u * DK, (u + 1) * DK) if PACK_SCORES else (0, DK)
                    nc.tensor.matmul(
                        sc[:, u * 512 : u * 512 + iw],
                        lhsT=kT_sb[lo:hi, t * 128 : (t + 1) * 128],
                        rhs=qT_sb[lo:hi, ioff : ioff + iw],
                        start=True,
                        stop=True,
                        tile_position=(lo, 0),
                    )
                ex = expp.tile([128, 1024], f16, tag="ex", name="ex")
                sc3 = sc.rearrange("p (b w) -> p b w", b=2)[:, :, :iw]
                ex3 = ex.rearrange("p (b w) -> p b w", b=2)[:, :, :iw]
                nc.scalar.activation(
                    out=ex3,
                    in_=sc3,
                    func=Exp,
                    bias=ebias_sb,
                    scale=0.125,
                )
                if g == 1 and pending_tail is not None:
                    pending_tail()
                    pending_tail = None
                pending_pv.append((ex, g))
                if len(pending_pv) > 1:
                    pex, pg = pending_pv.pop(0)
                    emit_pv(pv, pex, pg, iw)
            for pex, pg in pending_pv:
                emit_pv(pv, pex, pg, iw)
            res_sb = resp.tile([DK + 1, 512], fr, tag="res", name="res_sb")
            nc.vector.tensor_copy(res_sb[:, :iw], pv[:, :iw])
            nc.gpsimd.dma_start(
                out=lsum[0:1, ioff : ioff + iw],
                in_=res_sb[DK : DK + 1, :iw].bitcast(f32),
            )

            def tail(ioff=ioff, iw=iw, res_sb=res_sb):
                for cc in range(2):
                    po = psp.tile([128, 512], f32, tag="ps", name="po")
                    nc.tensor.matmul(
                        po[:, :iw],
                        lhsT=wo_sb[:, cc * 128 : (cc + 1) * 128],
                        rhs=res_sb[:DK, :iw],
                        start=True,
                        stop=True,
                    )
                    ob = outp.tile([128, 512], f32, tag="ob", name="ob")
                    nc.vector.tensor_copy(ob[:, :iw], po[:, :iw])
                    nc.sync.dma_start(
                        out=out[cc * 128 : (cc + 1) * 128, ioff : ioff + iw],
                        in_=ob[:, :iw],
                    )

            pending_tail = tail
        pending_tail()

    nc.compile()
    return nc


def _get_nc():
    global _NC
    if _NC is None:
        _NC = _build()
    return _NC



def _make_in_maps(inputs):
    x = np.asarray(inputs["x"], dtype=np.float32)
    w_proj = np.asarray(inputs["w_proj"], dtype=np.float32)
    b_proj = np.asarray(inputs["b_proj"], dtype=np.float32)
    w_out = np.asarray(inputs["w_out"], dtype=np.float32)
    in_maps = []
    for core in range(8):
        b, h = divmod(core, H)
        base = h * 3 * DK
        in_maps.append(
            {
                "xT": np.ascontiguousarray(x[b].reshape(C, S).astype(np.float16)),
                "wq": np.ascontiguousarray(
                    w_proj[:, base : base + DK].astype(np.float16)
                ),
                "wk": np.ascontiguousarray(
                    w_proj[:, base + DK : base + 2 * DK].astype(np.float16)
                ),
                "wv": np.ascontiguousarray(
                    w_proj[:, base + 2 * DK : base + 3 * DK].astype(np.float16)
                ),
                "bq": np.ascontiguousarray(
                    np.tile(b_proj[base : base + DK], 2).reshape(128, 1)
                ),
                "bk": np.ascontiguousarray(
                    np.tile(b_proj[base + DK : base + 2 * DK], 2).reshape(128, 1)
                ),
                "wo": np.ascontiguousarray(w_out[h * DK : (h + 1) * DK, :]),
            }
        )
    return in_maps


def kernel(x, w_proj, b_proj, w_out, b_out):
    from concourse.bass_utils import run_bass_kernel_spmd

    x = np.asarray(x, dtype=np.float32)
    w_proj = np.asarray(w_proj, dtype=np.float32)
    b_proj = np.asarray(b_proj, dtype=np.float32)
    w_out = np.asarray(w_out, dtype=np.float32)
    b_out = np.asarray(b_out, dtype=np.float32)

    B = x.shape[0]
    nc = _get_nc()

    in_maps = _make_in_maps(
        {"x": x, "w_proj": w_proj, "b_proj": b_proj, "w_out": w_out, "b_out": b_out}
    )
    res = run_bass_kernel_spmd(nc, in_maps, list(range(8)))

    outs = np.zeros((B, C, S), dtype=np.float32)
    for b in range(B):
        acc = x[b].reshape(C, S).astype(np.float32) + b_out[:, None]
        for h in range(H):
            core = b * H + h
            dev_o = res.results[core]["out"]  # [C, S] unnormalized
            l = res.results[core]["lsum"]  # [1, S]
            bv = b_proj[h * 3 * DK + 2 * DK : h * 3 * DK + 3 * DK]
            corr = bv @ w_out[h * DK : (h + 1) * DK, :]  # [C]
            acc = acc + dev_o / l + corr[:, None]
        outs[b] = acc
    return outs.reshape(B, C, 14, 14, 14)


# revision 24
# speedup vs baseline: 1.2571x; 1.2140x over previous
"""AttentionBlock kernel for 8 Trainium2 NeuronCores.

Sharding: one (batch, head) pair per core (B=2 x H=4 = 8 cores).
Each core computes, for its (b, h):
    qT/kT = (w_q/k^T @ x_b) + bias        [64, S]   (S padded to 2816)
    v     = x_b^T @ w_v                   [S, 64]  (+ ones column -> [S, 65])
    S^T[j, i] = sum_d k[j,d] q[i,d]                (22 j-tiles of 128)
    E = exp(S^T * 0.125 - 3)                       (ScalarE, from PSUM)
    resT[d, i] = sum_j v_aug[j, d] E[j, i]         (PSUM accumulation, 65 rows;
                                                    row 64 = softmax denominator l)
    outT[c, i] = sum_d w_out[d, c] resT[d, i]      [256, S] (unnormalized)
Host: out_b = sum_h (outT / l + (b_v @ w_out_h)[:, None]) + b_out[:, None] + x_b.

The -3 bias and missing max-subtraction cancel in the normalization; score
scale is 1/sqrt(64) = 0.125.

Matmul operands use float32r (single-pass PE fp32, ~TF32 precision, ~3x the
throughput of 2-pass fp32); PSUM accumulation stays fp32.
"""

import numpy as np

C = 256
S = 2744
SP = 2816  # 22 * 128
H = 4
DK = 64
NT = 22  # j tiles of 128
SVALID_LAST = S - 21 * 128  # 56 valid rows in last j-tile

# i blocks (query positions): only valid range [0, 2744)
IBLOCKS = [(0, 512), (512, 512), (1024, 512), (1536, 512), (2048, 512), (2560, 184)]
# s blocks for the qk projection: full padded range [0, 2816)
SBLOCKS = [(0, 512), (512, 512), (1024, 512), (1536, 512), (2048, 512), (2560, 256)]

_NC = None
PACK_SCORES = True


def _build():
    from contextlib import ExitStack

    import concourse.bacc as bacc
    import concourse.tile as tile
    from concourse import mybir

    f32 = mybir.dt.float32
    fr = mybir.dt.float32r
    f16 = mybir.dt.float16
    Exp = mybir.ActivationFunctionType.Exp

    nc = bacc.Bacc("TRN2", target_bir_lowering=False)

    xT = nc.dram_tensor("xT", [C, S], f16, kind="ExternalInput")
    wq = nc.dram_tensor("wq", [C, DK], f16, kind="ExternalInput")
    wk = nc.dram_tensor("wk", [C, DK], f16, kind="ExternalInput")
    wv = nc.dram_tensor("wv", [C, DK], f16, kind="ExternalInput")
    bq = nc.dram_tensor("bq", [128, 1], f32, kind="ExternalInput")
    bk = nc.dram_tensor("bk", [128, 1], f32, kind="ExternalInput")
    wo = nc.dram_tensor("wo", [DK, C], f32, kind="ExternalInput")

    out = nc.dram_tensor("out", [C, S], f32, kind="ExternalOutput")
    lsum = nc.dram_tensor("lsum", [1, S], f32, kind="ExternalOutput")

    with tile.TileContext(nc) as tc, ExitStack() as ctx:
        consts = ctx.enter_context(tc.tile_pool(name="consts", bufs=1))
        big = ctx.enter_context(tc.tile_pool(name="big", bufs=1))
        expp = ctx.enter_context(tc.tile_pool(name="expp", bufs=6))
        resp = ctx.enter_context(tc.tile_pool(name="resp", bufs=3))
        outp = ctx.enter_context(tc.tile_pool(name="outp", bufs=3))
        scp = ctx.enter_context(tc.tile_pool(name="scp", bufs=2, space="PSUM"))
        psp = ctx.enter_context(tc.tile_pool(name="psp", bufs=4, space="PSUM"))

        # ---- weights / constants in SBUF (fp16 direct) ----
        w_sb = consts.tile([128, 2, 3 * DK], f16)
        for idx, w_dram in enumerate((wq, wk, wv)):
            nc.gpsimd.dma_start(
                out=w_sb[:, :, idx * DK : (idx + 1) * DK],
                in_=w_dram.rearrange("(c p) d -> p c d", p=128),
            )

        def wslice(idx, cc):
            return w_sb[:, cc, idx * DK : (idx + 1) * DK]

        wo_stage = consts.tile([DK, C], f32)
        nc.gpsimd.dma_start(out=wo_stage, in_=wo[:, :])
        wo_sb = consts.tile([DK, C], fr)
        nc.vector.tensor_copy(wo_sb, wo_stage)

        bq_sb = consts.tile([128, 1], f32)
        nc.gpsimd.dma_start(out=bq_sb, in_=bq[:, :])
        bk_sb = consts.tile([128, 1], f32)
        nc.gpsimd.dma_start(out=bk_sb, in_=bk[:, :])
        ebias_sb = consts.tile([128, 1], f32)
        nc.vector.memset(ebias_sb, -3.0)

        # ---- x in SBUF (fp16 direct) ----
        x_sb = big.tile([128, 2, SP], f16)
        nc.vector.memset(x_sb[:, :, S:SP], 0.0)
        for off, w in SBLOCKS:
            for cc in range(2):
                wv_ = min(w, S - off) if off < S else 0
                if wv_ > 0:
                    eng = nc.sync if cc == 0 else nc.gpsimd
                    eng.dma_start(
                        out=x_sb[:, cc, off : off + wv_],
                        in_=xT[cc * 128 : (cc + 1) * 128, off : off + wv_],
                    )

        # ---- q/k/v projections, emitted as chunks interleaved into i-block 0 ----
        qT_sb = big.tile([128, SP], f16)
        kT_sb = big.tile([128, SP], f16)
        v_sb = big.tile([128, NT, DK + 1], f16)
        nc.vector.memset(v_sb[:, : NT - 1, DK : DK + 1], 1.0)
        nc.vector.memset(v_sb[:, NT - 1, DK : DK + 1], 0.0)
        nc.vector.memset(v_sb[:SVALID_LAST, NT - 1, DK : DK + 1], 1.0)

        def qk_chunk(sb):
            off, w = SBLOCKS[sb]
            for widx, dst, bias in ((0, qT_sb, bq_sb), (1, kT_sb, bk_sb)):
                ps = psp.tile([DK, 512], f32, tag="ps", name="psqk")
                for cc in range(2):
                    nc.tensor.matmul(
                        ps[:, :w],
                        lhsT=wslice(widx, cc),
                        rhs=x_sb[:, cc, off : off + w],
                        start=(cc == 0),
                        stop=(cc == 1),
                    )
                nc.vector.tensor_scalar_add(
                    dst[:DK, off : off + w], ps[:, :w], bias[:DK]
                )
                nc.gpsimd.dma_start(
                    out=dst[DK:, off : off + w], in_=dst[:DK, off : off + w]
                )

        def v_chunk(p):
            for t in (2 * p, 2 * p + 1):
                psv = psp.tile([128, DK], f32, tag="ps", name="psv")
                for cc in range(2):
                    nc.tensor.matmul(
                        psv,
                        lhsT=x_sb[:, cc, t * 128 : (t + 1) * 128],
                        rhs=wslice(2, cc),
                        start=(cc == 0),
                        stop=(cc == 1),
                    )
                nc.vector.tensor_copy(v_sb[:, t, :DK], psv)

        for sb in range(6):
            qk_chunk(sb)
        chunk_map = {
            g: [
                (lambda p=2 * g: v_chunk(p)),
                *([(lambda p=2 * g + 1: v_chunk(p))] if 2 * g + 1 < NT // 2 else []),
            ]
            for g in range(6)
        }

        # ---- main attention loop ----
        NG = NT // 2  # groups of 2 j-tiles per exp op

        def emit_pv(pv, pex, pg, iw):
            for u in range(2):
                t = 2 * pg + u
                nc.tensor.matmul(
                    pv[:, :iw],
                    lhsT=v_sb[:, t, :],
                    rhs=pex[:, u * 512 : u * 512 + iw],
                    start=(t == 0),
                    stop=(t == NT - 1),
                )

        pending_tail = None
        for ioff, iw in IBLOCKS:
            pv = psp.tile([DK + 1, 512], f32, tag="ps", name="pv")
            pending_pv = []
            for g in range(NG):
                if ioff == 0:
                    for ck in chunk_map.get(g, ()):
                        ck()
                sc = scp.tile([128, 1024], f32, tag="sc", name="sc")
                for u in range(2):
                    t = 2 * g + u
                    lo, hi = (u * DK, (u + 1) * DK) if PACK_SCORES else (0, DK)
                    nc.tensor.matmul(
                        sc[:, u * 512 : u * 512 + iw],
                        lhsT=kT_sb[lo:hi, t * 128 : (t + 1) * 128],
                        rhs=qT_sb[lo:hi, ioff : ioff + iw],
                        start=True,
                        stop=True,
                        tile_position=(lo, 0),
                    )
                ex = expp.tile([128, 1024], f16, tag="ex", name="ex")
                sc3 = sc.rearrange("p (b w) -> p b w", b=2)[:, :, :iw]
                ex3 = ex.rearrange("p (b w) -> p b w", b=2)[:, :, :iw]
                nc.scalar.activation(
                    out=ex3,
                    in_=sc3,
                    func=Exp,
                    bias=ebias_sb,
                    scale=0.125,
                )
                if g == 1 and pending_tail is not None:
                    pending_tail()
                    pending_tail = None
                pending_pv.append((ex, g))
                if len(pending_pv) > 1:
                    pex, pg = pending_pv.pop(0)
                    emit_pv(pv, pex, pg, iw)
            for pex, pg in pending_pv:
                emit_pv(pv, pex, pg, iw)
            res_sb = resp.tile([DK + 1, 512], fr, tag="res", name="res_sb")
            nc.vector.tensor_copy(res_sb[:, :iw], pv[:, :iw])
            nc.gpsimd.dma_start(
                out=lsum[0:1, ioff : ioff + iw],
                in_=res_sb[DK : DK + 1, :iw].bitcast(f32),
            )

            def tail(ioff=ioff, iw=iw, res_sb=res_sb):
                for cc in range(2):
                    po = psp.tile([128, 512], f32, tag="ps", name="po")
                    nc.tensor.matmul(
                        po[:, :iw],
                        lhsT=wo_sb[:, cc * 128 : (cc + 1) * 128],
                        rhs=res_sb[:DK, :iw],
                        start=True,
                        stop=True,
                    )
                    ob = outp.tile([128, 512], f32, tag="ob", name="ob")
                    nc.vector.tensor_copy(ob[:, :iw], po[:, :iw])
                    nc.sync.dma_start(
                        out=out[cc * 128 : (cc + 1) * 128, ioff : ioff + iw],
                        in_=ob[:, :iw],
                    )

            pending_tail = tail
        pending_tail()

    nc.compile()
    return nc


def _get_nc():
    global _NC
    if _NC is None:
        _NC = _build()
    return _NC



def _make_in_maps(inputs):
    x = np.asarray(inputs["x"], dtype=np.float32)
    w_proj = np.asarray(inputs["w_proj"], dtype=np.float32)
    b_proj = np.asarray(inputs["b_proj"], dtype=np.float32)
    w_out = np.asarray(inputs["w_out"], dtype=np.float32)
    in_maps = []
    for core in range(8):
        b, h = divmod(core, H)
        base = h * 3 * DK
        in_maps.append(
            {
                "xT": np.ascontiguousarray(x[b].reshape(C, S).astype(np.float16)),
                "wq": np.ascontiguousarray(
                    w_proj[:, base : base + DK].astype(np.float16)
                ),
                "wk": np.ascontiguousarray(
                    w_proj[:, base + DK : base + 2 * DK].astype(np.float16)
                ),
                "wv": np.ascontiguousarray(
                    w_proj[:, base + 2 * DK : base + 3 * DK].astype(np.float16)
                ),
                "bq": np.ascontiguousarray(
                    np.tile(b_proj[base : base + DK], 2).reshape(128, 1)
                ),
                "bk": np.ascontiguousarray(
                    np.tile(b_proj[base + DK : base + 2 * DK], 2).reshape(128, 1)
                ),
                "wo": np.ascontiguousarray(w_out[h * DK : (h + 1) * DK, :]),
            }
        )
    return in_maps


def kernel(x, w_proj, b_proj, w_out, b_out):
    from concourse.bass_utils import run_bass_kernel_spmd

    x = np.asarray(x, dtype=np.float32)
    w_proj = np.asarray(w_proj, dtype=np.float32)
    b_proj = np.asarray(b_proj, dtype=np.float32)
    w_out = np.asarray(w_out, dtype=np.float32)
    b_out = np.asarray(b_out, dtype=np.float32)

    B = x.shape[0]
    nc = _get_nc()

    in_maps = _make_in_maps(
        {"x": x, "w_proj": w_proj, "b_proj": b_proj, "w_out": w_out, "b_out": b_out}
    )
    res = run_bass_kernel_spmd(nc, in_maps, list(range(8)))

    outs = np.zeros((B, C, S), dtype=np.float32)
    for b in range(B):
        acc = x[b].reshape(C, S).astype(np.float32) + b_out[:, None]
        for h in range(H):
            core = b * H + h
            dev_o = res.results[core]["out"]  # [C, S] unnormalized
            l = res.results[core]["lsum"]  # [1, S]
            bv = b_proj[h * 3 * DK + 2 * DK : h * 3 * DK + 3 * DK]
            corr = bv @ w_out[h * DK : (h + 1) * DK, :]  # [C]
            acc = acc + dev_o / l + corr[:, None]
        outs[b] = acc
    return outs.reshape(B, C, 14, 14, 14)
